# revision 16
# baseline (speedup 1.0000x reference)
"""Trainium2 Bass kernel for BaseSSMLayer (diagonal linear SSM).

Computation (exactly equivalent to the reference's associative_scan — for
broadcast lambda the non-standard cell reduces to the standard recurrence):
    U = xs @ w_in.T              # [L, N]
    h_t = lam * h_{t-1} + U_t    # linear recurrence over L
    Y = H @ c_out.T + xs * d_skip

Sharding: 4 time blocks x 2 state halves (8 cores).  Each core owns a
4096-step block and 1024 state channels (globally lambda-sorted, split into
two halves, each half sorted so only its top group g7 has long memory).
Per-core DMA is ~44 MB (vs 137 MB for pure state sharding), leaving the DMA
engines ~70% idle so transient HBM contention can't starve the PE — the
PE window is then just the 2 GEMMs at the bf16 roofline.

Cross-block state: each core runs a zero-init local scan.  The incoming
state h_init = sum_{q'<q} Lam^(Q(q-1-q')) F(q') needs the other time blocks'
final local states F, exchanged mid-kernel via an 8-core AllGather (8 KiB).
The scan superposition  h_true(d) = h_local(d) + lam^(d+1) * h_init  makes
the correction purely elementwise.  Corrections only matter for tile tau=0
(all groups, small d) and the hot group g7 (lambda in [~0.94, 1)) on later
tiles: lambda^512 < 2e-17 for every cold group.  The mm2 schedule exploits
that: phase B1 (cold groups, tiles 1..7) runs right after mm1 with no
barrier, giving the collective a ~170 us window to complete; B2 (hot group,
corrected) and the tau=0 tile run at the end.  B2's contribution goes to a
separate ycorr output summed on the host, so no PSUM group ever spans the
barrier.
"""

import numpy as np
import ml_dtypes

import concourse.tile as tile
from concourse import bacc, mybir
from concourse.bass import ts
from concourse.bass_utils import run_bass_kernel_spmd

L = 16384        # sequence length
I = 2048         # in_dim (= out dim of Y)
N = 2048         # state_dim
NCORES = 8
NQ = 4           # time blocks
NS = 2           # state halves
Q = L // NQ      # 4096 timesteps per block
NCH = N // NS    # 1024 channels per core
NGQ = NCH // 128  # 8 channel groups per core
TT = 512         # time tile
NTQ = Q // TT    # 8 time tiles per block
KI = I // 128    # 16 contraction chunks over in_dim
XC = 4           # x DMA chunking: KI split into XC chunks of KXC i-tiles
KXC = KI // XC
HOT = NGQ - 1    # index of the hot (long-memory) group

BF16 = mybir.dt.bfloat16
F32 = mybir.dt.float32
NP_BF16 = ml_dtypes.bfloat16


def _build_nc():
    nc = bacc.Bacc(
        "TRN2",
        target_bir_lowering=False,
        debug=False,
        num_devices=NCORES,
    )
    xt = nc.dram_tensor("xt", [NTQ, 128, KI * TT], BF16, kind="ExternalInput").ap()
    wt = nc.dram_tensor("wt", [I, NCH], BF16, kind="ExternalInput").ap()
    ct = nc.dram_tensor("ct", [NCH, I], BF16, kind="ExternalInput").ap()
    lamb = nc.dram_tensor("lamb", [128, NGQ * TT], F32, kind="ExternalInput").ap()
    lpw = nc.dram_tensor("lpw", [128, (NGQ + NTQ - 1) * TT], BF16, kind="ExternalInput").ap()
    dcoef = nc.dram_tensor("dcoef", [128, NCORES * NGQ], F32, kind="ExternalInput").ap()
    y = nc.dram_tensor("y", [NTQ, 128, KI * TT], BF16, kind="ExternalOutput").ap()
    ycorr = nc.dram_tensor("ycorr", [NTQ - 1, 128, KI * TT], BF16, kind="ExternalOutput").ap()

    with tile.TileContext(nc) as tc:
        with (
            tc.tile_pool(name="const", bufs=1) as const_pool,
            tc.tile_pool(name="xin", bufs=6) as x_pool,
            tc.tile_pool(name="hb", bufs=1) as hb_pool,
            tc.tile_pool(name="yst", bufs=4) as yst_pool,
            tc.tile_pool(name="sm", bufs=8) as sm_pool,
            tc.tile_pool(name="dram", bufs=2, space="DRAM") as dram_pool,
            tc.tile_pool(name="ups", bufs=2, space="PSUM") as u_psum,
            tc.tile_pool(name="yps", bufs=4, space="PSUM") as y_psum,
        ):
            # --- resident constants ---
            w_sb = []
            for i in range(KI):
                w = const_pool.tile([128, NCH], BF16, tag=f"w{i}", name=f"w{i}")
                nc.scalar.dma_start(w[:], wt[i * 128:(i + 1) * 128, :])
                w_sb.append(w)
            lam_sb = const_pool.tile([128, NGQ * TT], F32, tag="lam")
            nc.scalar.dma_start(lam_sb[:], lamb[:])
            dco_sb = const_pool.tile([128, NCORES * NGQ], F32, tag="dco")
            nc.scalar.dma_start(dco_sb[:], dcoef[:])
            lpw_sb = const_pool.tile([128, (NGQ + NTQ - 1) * TT], BF16, tag="lpw")
            nc.scalar.dma_start(lpw_sb[:], lpw[:])
            c_sb = []
            for g in range(NGQ):
                c = const_pool.tile([128, I], BF16, tag=f"c{g}", name=f"c{g}")
                nc.scalar.dma_start(c[:], ct[g * 128:(g + 1) * 128, :])
                c_sb.append(c)

            # Pre-warm the PE HAM clock gate during the initial DMA ramp.
            warm_sb = const_pool.tile([128, 128], BF16, tag="warm")
            nc.gpsimd.memset(warm_sb[:], 0.0)
            warm_ps = u_psum.tile([128, 128], F32, tag="u")
            for _ in range(16):
                nc.tensor.matmul(warm_ps[:], warm_sb[:], warm_sb[:], start=True, stop=True)

            xt3 = xt.rearrange("t p (j c) -> t p j c", j=XC)
            yo4 = y.rearrange("t p (o u) -> t p o u", o=KI)
            yc4 = ycorr.rearrange("t p (o u) -> t p o u", o=KI)

            hb_tiles = [[None] * NGQ for _ in range(NTQ)]
            hb_prev = [None] * NGQ

            # ---------------- phase A: mm1 + scans ----------------
            for t in range(NTQ):
                x_chunks = []
                for j in range(XC):
                    xc = x_pool.tile([128, KXC * TT], BF16, tag="x")
                    nc.sync.dma_start(xc[:], xt3[t, :, j])
                    x_chunks.append(xc)
                for g in range(NGQ):
                    u_ps = u_psum.tile([128, TT], F32, tag="u")
                    for i in range(KI):
                        j, ic = divmod(i, KXC)
                        nc.tensor.matmul(
                            u_ps[:],
                            w_sb[i][:, g * 128:(g + 1) * 128],
                            x_chunks[j][:, ts(ic, TT)],
                            start=(i == 0),
                            stop=(i == KI - 1),
                        )
                        if t == 0 and g == 0 and i % 4 == 3 and i < KI - 1:
                            # keep the PE busy through the DMA-paced ramp
                            for _ in range(2):
                                nc.tensor.matmul(
                                    warm_ps[:], warm_sb[:], warm_sb[:],
                                    start=True, stop=True,
                                )
                    hb = hb_pool.tile([128, TT], BF16, tag="hb", bufs=NTQ * NGQ + 2)
                    init = 0.0 if t == 0 else hb_prev[g][:, TT - 1: TT]
                    nc.vector.tensor_tensor_scan(
                        hb[:],
                        lam_sb[:, ts(g, TT)],
                        u_ps[:],
                        init,
                        op0=mybir.AluOpType.mult,
                        op1=mybir.AluOpType.add,
                    )
                    hb_prev[g] = hb
                    hb_tiles[t][g] = hb

            # ---------------- F exchange (gpsimd queue, overlaps B1) ----------
            f_sb = const_pool.tile([128, NGQ], F32, tag="fsb")
            for g in range(NGQ):
                nc.vector.tensor_copy(f_sb[:, g:g + 1], hb_tiles[NTQ - 1][g][:, TT - 1: TT])
            cc_in = dram_pool.tile([128, NGQ], F32, tag="ccin")
            cc_out = dram_pool.tile([NCORES, 128, NGQ], F32, tag="ccout")
            nc.gpsimd.dma_start(cc_in[:], f_sb[:])
            nc.gpsimd.collective_compute(
                "AllGather",
                mybir.AluOpType.bypass,
                replica_groups=[list(range(NCORES))],
                ins=[cc_in.opt()],
                outs=[cc_out.opt()],
            )
            gath_sb = const_pool.tile([128, NCORES * NGQ], F32, tag="gath")
            for r in range(NCORES):
                nc.gpsimd.dma_start(gath_sb[:, ts(r, NGQ)], cc_out[r])
            # h_init[p, g] = sum_r gath[r, p, g] * dcoef[r, p, g]
            acc = sm_pool.tile([128, NGQ], F32, tag="hacc")
            nc.vector.tensor_tensor(
                acc[:], gath_sb[:, ts(0, NGQ)], dco_sb[:, ts(0, NGQ)],
                op=mybir.AluOpType.mult,
            )
            for r in range(1, NCORES):
                prod = sm_pool.tile([128, NGQ], F32, tag="hprod")
                nc.vector.tensor_tensor(
                    prod[:], gath_sb[:, ts(r, NGQ)], dco_sb[:, ts(r, NGQ)],
                    op=mybir.AluOpType.mult,
                )
                acc2 = sm_pool.tile([128, NGQ], F32, tag="hacc")
                nc.vector.tensor_tensor(
                    acc2[:], acc[:], prod[:], op=mybir.AluOpType.add,
                )
                acc = acc2
            h_init = acc

            # corrections: hbc = lpw * h_init + hb_local   (gpsimd, overlaps B1)
            hbc0 = [None] * NGQ     # tau=0, all groups
            for g in range(NGQ):
                hc = hb_pool.tile([128, TT], BF16, tag="hbc", bufs=NGQ + NTQ)
                nc.vector.scalar_tensor_tensor(
                    hc[:],
                    lpw_sb[:, ts(g, TT)],
                    h_init[:, g:g + 1],
                    hb_tiles[0][g][:],
                    op0=mybir.AluOpType.mult,
                    op1=mybir.AluOpType.add,
                )
                hbc0[g] = hc
            hbc7 = [None] * NTQ     # tau=1..7, hot group
            for t in range(1, NTQ):
                hc = hb_pool.tile([128, TT], BF16, tag="hbc", bufs=NGQ + NTQ)
                nc.vector.scalar_tensor_tensor(
                    hc[:],
                    lpw_sb[:, ts(NGQ + t - 1, TT)],
                    h_init[:, HOT:HOT + 1],
                    hb_tiles[t][HOT][:],
                    op0=mybir.AluOpType.mult,
                    op1=mybir.AluOpType.add,
                )
                hbc7[t] = hc

            # ---------------- phase B1: cold groups, tiles 1..7 ----------------
            def emit_b1(t):
                for o in range(KI):
                    y_ps = y_psum.tile([128, TT], F32, tag="y")
                    for g in range(NGQ - 1):
                        nc.tensor.matmul(
                            y_ps[:],
                            c_sb[g][:, ts(o, 128)],
                            hb_tiles[t][g][:],
                            start=(g == 0),
                            stop=(g == NGQ - 2),
                        )
                    y_stage = yst_pool.tile([128, TT], BF16, tag="yst")
                    if o % 2 == 0:
                        nc.vector.tensor_copy(y_stage[:], y_ps[:])
                    else:
                        nc.scalar.copy(y_stage[:], y_ps[:])
                    nc.scalar.dma_start(yo4[t, :, o], y_stage[:])

            # ---------------- phase B2: hot group (corrected), tiles 1..7 ------
            def emit_b2(t):
                for o in range(KI):
                    y_ps = y_psum.tile([128, TT], F32, tag="y")
                    nc.tensor.matmul(
                        y_ps[:],
                        c_sb[HOT][:, ts(o, 128)],
                        hbc7[t][:],
                        start=True,
                        stop=True,
                    )
                    y_stage = yst_pool.tile([128, TT], BF16, tag="yst")
                    if o % 2 == 0:
                        nc.vector.tensor_copy(y_stage[:], y_ps[:])
                    else:
                        nc.scalar.copy(y_stage[:], y_ps[:])
                    nc.scalar.dma_start(yc4[t - 1, :, o], y_stage[:])

            for t in range(1, 5):
                emit_b1(t)
            for t in range(1, NTQ):
                emit_b2(t)
            for t in range(5, NTQ):
                emit_b1(t)

            # ---------------- tail: tile 0 (fully corrected) -------------------
            for o in range(KI):
                y_ps = y_psum.tile([128, TT], F32, tag="y")
                for g in range(NGQ):
                    nc.tensor.matmul(
                        y_ps[:],
                        c_sb[g][:, ts(o, 128)],
                        hbc0[g][:],
                        start=(g == 0),
                        stop=(g == NGQ - 1),
                    )
                y_stage = yst_pool.tile([128, TT], BF16, tag="yst")
                if o % 2 == 0:
                    nc.vector.tensor_copy(y_stage[:], y_ps[:])
                else:
                    nc.scalar.copy(y_stage[:], y_ps[:])
                nc.scalar.dma_start(yo4[0, :, o], y_stage[:])

    nc.compile()
    return nc


_NC_CACHE = None


def _get_nc():
    global _NC_CACHE
    if _NC_CACHE is None:
        _NC_CACHE = _build_nc()
    return _NC_CACHE


_PREP_CACHE = {}


def _prep_in_maps(xs, lam, w_in, c_out):
    """Per-core inputs for the 4x2 time/state sharding."""
    # channel assignment: global lambda sort, split halves, each half ascending
    order = np.argsort(lam, kind="stable")
    halves = [order[:NCH], order[NCH:]]

    # x slabs per time block (shared by the two state-half cores of a block)
    x_slabs = []
    for q in range(NQ):
        blk = np.ascontiguousarray(xs[q * Q:(q + 1) * Q].T)    # [I, Q]
        slab = (
            blk.astype(NP_BF16)
            .reshape(KI, 128, NTQ, TT)
            .transpose(2, 1, 0, 3)
            .reshape(NTQ, 128, KI * TT)
        )
        x_slabs.append(np.ascontiguousarray(slab))

    w_t = np.ascontiguousarray(w_in.T)    # [I, N]
    c_t = np.ascontiguousarray(c_out.T)   # [N, I]

    delta = np.arange(TT, dtype=np.float64)
    in_maps = []
    for k in range(NCORES):
        q, s = divmod(k, NS)
        ch = halves[s]
        lm = lam[ch].astype(np.float64)                        # [NCH]
        wt = np.ascontiguousarray(w_t[:, ch]).astype(NP_BF16)  # [I, NCH]
        ctm = np.ascontiguousarray(c_t[ch, :]).astype(NP_BF16)  # [NCH, I]
        lm_g = lm.reshape(NGQ, 128)                            # [g, p]
        lamb = np.ascontiguousarray(
            np.broadcast_to(lm_g[:, :, None], (NGQ, 128, TT))
            .transpose(1, 0, 2)
            .reshape(128, NGQ * TT)
            .astype(np.float32)
        )
        # lpw columns: g*TT+u -> lam_{g,p}^(u+1)  (tau=0 corrections)
        #              (NGQ+t-1)*TT+u -> lam_{HOT,p}^(t*TT+u+1)  (hot group)
        lpw = np.empty((128, (NGQ + NTQ - 1) * TT), np.float32)
        for g in range(NGQ):
            lpw[:, g * TT:(g + 1) * TT] = lm_g[g][:, None] ** (delta[None, :] + 1)
        for t in range(1, NTQ):
            lpw[:, (NGQ + t - 1) * TT:(NGQ + t) * TT] = (
                lm_g[HOT][:, None] ** (t * TT + delta[None, :] + 1)
            )
        lpw = lpw.astype(NP_BF16)
        # dcoef[p, r*NGQ+g] = lam_{g,p}^(Q*(q-1-q_r)) if s_r==s and q_r<q else 0
        dco = np.zeros((128, NCORES * NGQ), np.float64)
        for r in range(NCORES):
            qr, sr = divmod(r, NS)
            if sr == s and qr < q:
                dco[:, r * NGQ:(r + 1) * NGQ] = (lm_g ** (Q * (q - 1 - qr))).T
        in_maps.append({
            "xt": x_slabs[q],
            "wt": wt,
            "ct": ctm,
            "lamb": lamb,
            "lpw": np.ascontiguousarray(lpw),
            "dcoef": np.ascontiguousarray(dco.astype(np.float32)),
        })
    return in_maps


def _unslab(slab, ntq):
    """[ntq, 128, KI*TT] -> y_T [I, ntq*TT] f32."""
    a = slab.astype(np.float32).reshape(ntq, 128, KI, TT)
    return a.transpose(2, 1, 0, 3).reshape(I, ntq * TT)


def combine_outputs(results, xs, d_skip):
    """results: per-core {"y", "ycorr"} -> full Y [L, I] f32."""
    out = np.empty((L, I), np.float32)
    for q in range(NQ):
        acc = None
        for s in range(NS):
            r = results[q * NS + s]
            yt = _unslab(r["y"], NTQ)                       # [I, Q]
            yt[:, TT:] += _unslab(r["ycorr"], NTQ - 1)
            acc = yt if acc is None else acc + yt
        out[q * Q:(q + 1) * Q] = acc.T
    out += xs * d_skip[None, :].astype(np.float32)
    return np.ascontiguousarray(out, dtype=np.float32)


def run_on_hw(xs, lam, w_in, c_out, d_skip):
    """Returns (Y full f32 [L, I], BassKernelResults)."""
    nc = _get_nc()
    in_maps = _prep_in_maps(xs, lam, w_in, c_out)
    res = run_bass_kernel_spmd(nc, in_maps, core_ids=list(range(NCORES)))
    return combine_outputs(res.results, xs, d_skip), res


def kernel(xs, lam, w_in, c_out, d_skip):
    out, _ = run_on_hw(
        np.asarray(xs, dtype=np.float32),
        np.asarray(lam, dtype=np.float32),
        np.asarray(w_in, dtype=np.float32),
        np.asarray(c_out, dtype=np.float32),
        np.asarray(d_skip, dtype=np.float32),
    )
    return out


# revision 19
# speedup vs baseline: 1.1171x; 1.1171x over previous
"""Trainium2 Bass kernel for BaseSSMLayer (diagonal linear SSM).

Computation (exactly equivalent to the reference's associative_scan — for
broadcast lambda the non-standard cell reduces to the standard recurrence):
    U = xs @ w_in.T              # [L, N]
    h_t = lam * h_{t-1} + U_t    # linear recurrence over L
    Y = H @ c_out.T + xs * d_skip

Sharding: 4 time blocks x 2 state halves (8 cores).  Each core owns a
4096-step block and 1024 state channels (globally lambda-sorted, split into
two halves, each half sorted so only its top group g7 has long memory).
Per-core DMA is ~44 MB (vs 137 MB for pure state sharding), leaving the DMA
engines ~70% idle so transient HBM contention can't starve the PE — the
PE window is then just the 2 GEMMs at the bf16 roofline.

Cross-block state: each core runs a zero-init local scan.  The incoming
state h_init = sum_{q'<q} Lam^(Q(q-1-q')) F(q') needs the other time blocks'
final local states F, exchanged mid-kernel via an 8-core AllGather (8 KiB).
The scan superposition  h_true(d) = h_local(d) + lam^(d+1) * h_init  makes
the correction purely elementwise.  Corrections only matter for tile tau=0
(all groups, small d) and the hot group g7 (lambda in [~0.94, 1)) on later
tiles: lambda^512 < 2e-17 for every cold group.  The mm2 schedule exploits
that: phase B1 (cold groups, tiles 1..7) runs right after mm1 with no
barrier, giving the collective a ~170 us window to complete; B2 (hot group,
corrected) and the tau=0 tile run at the end.  B2's contribution goes to a
separate ycorr output summed on the host, so no PSUM group ever spans the
barrier.
"""

import numpy as np
import ml_dtypes

import concourse.tile as tile
from concourse import bacc, mybir
from concourse.bass import ts
from concourse.bass_utils import run_bass_kernel_spmd

L = 16384        # sequence length
I = 2048         # in_dim (= out dim of Y)
N = 2048         # state_dim
NCORES = 8
NQ = 4           # time blocks
NS = 2           # state halves
Q = L // NQ      # 4096 timesteps per block
NCH = N // NS    # 1024 channels per core
NGQ = NCH // 128  # 8 channel groups per core
TT = 512         # time tile
NTQ = Q // TT    # 8 time tiles per block
KI = I // 128    # 16 contraction chunks over in_dim
XC = 4           # x DMA chunking: KI split into XC chunks of KXC i-tiles
KXC = KI // XC
HOT = NGQ - 1    # index of the hot (long-memory) group

BF16 = mybir.dt.bfloat16
F32 = mybir.dt.float32
NP_BF16 = ml_dtypes.bfloat16


def _build_nc():
    nc = bacc.Bacc(
        "TRN2",
        target_bir_lowering=False,
        debug=False,
        num_devices=NCORES,
    )
    xt = nc.dram_tensor("xt", [NTQ, 128, KI * TT], BF16, kind="ExternalInput").ap()
    wt = nc.dram_tensor("wt", [I, NCH], BF16, kind="ExternalInput").ap()
    ct = nc.dram_tensor("ct", [NCH, I], BF16, kind="ExternalInput").ap()
    lamb = nc.dram_tensor("lamb", [128, NGQ * TT], F32, kind="ExternalInput").ap()
    lpw = nc.dram_tensor("lpw", [128, (NGQ + NTQ - 1) * TT], BF16, kind="ExternalInput").ap()
    dcoef = nc.dram_tensor("dcoef", [128, NCORES * NGQ], F32, kind="ExternalInput").ap()
    y = nc.dram_tensor("y", [NTQ, 128, KI * TT], BF16, kind="ExternalOutput").ap()
    ycorr = nc.dram_tensor("ycorr", [NTQ - 1, 128, KI * TT], BF16, kind="ExternalOutput").ap()

    with tile.TileContext(nc) as tc:
        with (
            tc.tile_pool(name="const", bufs=1) as const_pool,
            tc.tile_pool(name="xin", bufs=5) as x_pool,
            tc.tile_pool(name="hb", bufs=1) as hb_pool,
            tc.tile_pool(name="yst", bufs=4) as yst_pool,
            tc.tile_pool(name="sm", bufs=8) as sm_pool,
            tc.tile_pool(name="dram", bufs=2, space="DRAM") as dram_pool,
            tc.tile_pool(name="ups", bufs=2, space="PSUM") as u_psum,
            tc.tile_pool(name="yps", bufs=4, space="PSUM") as y_psum,
        ):
            # --- resident constants ---
            w_sb = []
            for i in range(KI):
                w = const_pool.tile([128, NCH], BF16, tag=f"w{i}", name=f"w{i}")
                nc.scalar.dma_start(w[:], wt[i * 128:(i + 1) * 128, :])
                w_sb.append(w)
            lam_sb = const_pool.tile([128, NGQ * TT], F32, tag="lam")
            nc.scalar.dma_start(lam_sb[:], lamb[:])
            dco_sb = const_pool.tile([128, NCORES * NGQ], F32, tag="dco")
            nc.scalar.dma_start(dco_sb[:], dcoef[:])
            lpw_sb = const_pool.tile([128, (NGQ + NTQ - 1) * TT], BF16, tag="lpw")
            nc.scalar.dma_start(lpw_sb[:], lpw[:])
            c_sb = []
            for g in range(NGQ):
                c = const_pool.tile([128, I], BF16, tag=f"c{g}", name=f"c{g}")
                nc.scalar.dma_start(c[:], ct[g * 128:(g + 1) * 128, :])
                c_sb.append(c)

            # Pre-warm the PE HAM clock gate during the initial DMA ramp.
            warm_sb = const_pool.tile([128, 128], BF16, tag="warm")
            nc.gpsimd.memset(warm_sb[:], 0.0)
            warm_ps = u_psum.tile([128, 128], F32, tag="u")
            for _ in range(16):
                nc.tensor.matmul(warm_ps[:], warm_sb[:], warm_sb[:], start=True, stop=True)

            xt3 = xt.rearrange("t p (j c) -> t p j c", j=XC)
            yo4 = y.rearrange("t p (o u) -> t p o u", o=KI)
            yc4 = ycorr.rearrange("t p (o u) -> t p o u", o=KI)

            hb_tiles = [[None] * NGQ for _ in range(NTQ)]
            hb_prev = [None] * NGQ

            # ---------------- phase A: mm1 + scans ----------------
            for t in range(NTQ):
                x_chunks = []
                for j in range(XC):
                    xc = x_pool.tile([128, KXC * TT], BF16, tag="x")
                    nc.sync.dma_start(xc[:], xt3[t, :, j])
                    x_chunks.append(xc)
                for g in range(NGQ):
                    u_ps = u_psum.tile([128, TT], F32, tag="u")
                    for i in range(KI):
                        j, ic = divmod(i, KXC)
                        nc.tensor.matmul(
                            u_ps[:],
                            w_sb[i][:, g * 128:(g + 1) * 128],
                            x_chunks[j][:, ts(ic, TT)],
                            start=(i == 0),
                            stop=(i == KI - 1),
                        )
                        if (t == 0 or (t == 1 and g < 4)) and i % 4 == 3 and i < KI - 1:
                            # keep the PE busy through the DMA-paced ramp so
                            # the HAM clock gate never re-throttles
                            for _ in range(2):
                                nc.tensor.matmul(
                                    warm_ps[:], warm_sb[:], warm_sb[:],
                                    start=True, stop=True,
                                )
                    hb = hb_pool.tile([128, TT], BF16, tag="hb", bufs=NTQ * NGQ + 2)
                    init = 0.0 if t == 0 else hb_prev[g][:, TT - 1: TT]
                    nc.vector.tensor_tensor_scan(
                        hb[:],
                        lam_sb[:, ts(g, TT)],
                        u_ps[:],
                        init,
                        op0=mybir.AluOpType.mult,
                        op1=mybir.AluOpType.add,
                    )
                    hb_prev[g] = hb
                    hb_tiles[t][g] = hb

            # ---------------- F exchange (gpsimd queue, overlaps B1) ----------
            f_sb = const_pool.tile([128, NGQ], F32, tag="fsb")
            for g in range(NGQ):
                nc.vector.tensor_copy(f_sb[:, g:g + 1], hb_tiles[NTQ - 1][g][:, TT - 1: TT])
            cc_in = dram_pool.tile([128, NGQ], F32, tag="ccin")
            cc_out = dram_pool.tile([NCORES, 128, NGQ], F32, tag="ccout")
            nc.gpsimd.dma_start(cc_in[:], f_sb[:])
            nc.gpsimd.collective_compute(
                "AllGather",
                mybir.AluOpType.bypass,
                replica_groups=[list(range(NCORES))],
                ins=[cc_in.opt()],
                outs=[cc_out.opt()],
            )
            gath_sb = const_pool.tile([128, NCORES * NGQ], F32, tag="gath")
            for r in range(NCORES):
                nc.gpsimd.dma_start(gath_sb[:, ts(r, NGQ)], cc_out[r])
            # h_init[p, g] = sum_r gath[r, p, g] * dcoef[r, p, g]
            acc = sm_pool.tile([128, NGQ], F32, tag="hacc")
            nc.vector.tensor_tensor(
                acc[:], gath_sb[:, ts(0, NGQ)], dco_sb[:, ts(0, NGQ)],
                op=mybir.AluOpType.mult,
            )
            for r in range(1, NCORES):
                prod = sm_pool.tile([128, NGQ], F32, tag="hprod")
                nc.vector.tensor_tensor(
                    prod[:], gath_sb[:, ts(r, NGQ)], dco_sb[:, ts(r, NGQ)],
                    op=mybir.AluOpType.mult,
                )
                acc2 = sm_pool.tile([128, NGQ], F32, tag="hacc")
                nc.vector.tensor_tensor(
                    acc2[:], acc[:], prod[:], op=mybir.AluOpType.add,
                )
                acc = acc2
            h_init = acc

            # corrections: hbc = lpw * h_init + hb_local   (gpsimd, overlaps B1)
            hbc0 = [None] * NGQ     # tau=0, all groups
            for g in range(NGQ):
                hc = hb_pool.tile([128, TT], BF16, tag="hbc", bufs=NGQ + NTQ - 1)
                nc.vector.scalar_tensor_tensor(
                    hc[:],
                    lpw_sb[:, ts(g, TT)],
                    h_init[:, g:g + 1],
                    hb_tiles[0][g][:],
                    op0=mybir.AluOpType.mult,
                    op1=mybir.AluOpType.add,
                )
                hbc0[g] = hc
            hbc7 = [None] * NTQ     # tau=1..7, hot group
            for t in range(1, NTQ):
                hc = hb_pool.tile([128, TT], BF16, tag="hbc", bufs=NGQ + NTQ - 1)
                nc.vector.scalar_tensor_tensor(
                    hc[:],
                    lpw_sb[:, ts(NGQ + t - 1, TT)],
                    h_init[:, HOT:HOT + 1],
                    hb_tiles[t][HOT][:],
                    op0=mybir.AluOpType.mult,
                    op1=mybir.AluOpType.add,
                )
                hbc7[t] = hc

            # ---------------- phase B1: cold groups, tiles 1..7 ----------------
            # Engine split avoids head-of-line blocking: ACT drains B1/tau0
            # groups (never waits on the collective), DVE drains B2 (queued
            # after h_init, which is where B2 can first run anyway), and the
            # idle-by-then sync queue issues all y DMAs.
            def emit_b1(t, b2_iter=None):
                for o in range(KI):
                    y_ps = y_psum.tile([128, TT], F32, tag="y")
                    for g in range(NGQ - 1):
                        nc.tensor.matmul(
                            y_ps[:],
                            c_sb[g][:, ts(o, 128)],
                            hb_tiles[t][g][:],
                            start=(g == 0),
                            stop=(g == NGQ - 2),
                        )
                    y_stage = yst_pool.tile([128, TT], BF16, tag="yst")
                    nc.scalar.copy(y_stage[:], y_ps[:])
                    nc.sync.dma_start(yo4[t, :, o], y_stage[:])
                    if b2_iter is not None:
                        for _ in range(3):
                            item = next(b2_iter, None)
                            if item is not None:
                                emit_b2_one(*item)

            # ---------------- phase B2: hot group (corrected), tiles 1..7 ------
            def emit_b2_one(t, o):
                y_ps = y_psum.tile([128, TT], F32, tag="y")
                nc.tensor.matmul(
                    y_ps[:],
                    c_sb[HOT][:, ts(o, 128)],
                    hbc7[t][:],
                    start=True,
                    stop=True,
                )
                y_stage = yst_pool.tile([128, TT], BF16, tag="ystc")
                nc.vector.tensor_copy(y_stage[:], y_ps[:])
                nc.sync.dma_start(yc4[t - 1, :, o], y_stage[:])

            for t in range(1, 5):
                emit_b1(t)
            b2_iter = iter([(t, o) for t in range(1, NTQ) for o in range(KI)])
            for t in range(5, NTQ):
                emit_b1(t, b2_iter)
            for item in b2_iter:
                emit_b2_one(*item)

            # ---------------- tail: tile 0 (fully corrected) -------------------
            for o in range(KI):
                y_ps = y_psum.tile([128, TT], F32, tag="y")
                for g in range(NGQ):
                    nc.tensor.matmul(
                        y_ps[:],
                        c_sb[g][:, ts(o, 128)],
                        hbc0[g][:],
                        start=(g == 0),
                        stop=(g == NGQ - 1),
                    )
                y_stage = yst_pool.tile([128, TT], BF16, tag="yst")
                nc.scalar.copy(y_stage[:], y_ps[:])
                nc.sync.dma_start(yo4[0, :, o], y_stage[:])

    nc.compile()
    return nc


_NC_CACHE = None


def _get_nc():
    global _NC_CACHE
    if _NC_CACHE is None:
        _NC_CACHE = _build_nc()
    return _NC_CACHE


_PREP_CACHE = {}


def _prep_in_maps(xs, lam, w_in, c_out):
    """Per-core inputs for the 4x2 time/state sharding."""
    # channel assignment: global lambda sort, split halves, each half ascending
    order = np.argsort(lam, kind="stable")
    halves = [order[:NCH], order[NCH:]]

    # x slabs per time block (shared by the two state-half cores of a block)
    x_slabs = []
    for q in range(NQ):
        blk = np.ascontiguousarray(xs[q * Q:(q + 1) * Q].T)    # [I, Q]
        slab = (
            blk.astype(NP_BF16)
            .reshape(KI, 128, NTQ, TT)
            .transpose(2, 1, 0, 3)
            .reshape(NTQ, 128, KI * TT)
        )
        x_slabs.append(np.ascontiguousarray(slab))

    w_t = np.ascontiguousarray(w_in.T)    # [I, N]
    c_t = np.ascontiguousarray(c_out.T)   # [N, I]

    delta = np.arange(TT, dtype=np.float64)
    in_maps = []
    for k in range(NCORES):
        q, s = divmod(k, NS)
        ch = halves[s]
        lm = lam[ch].astype(np.float64)                        # [NCH]
        wt = np.ascontiguousarray(w_t[:, ch]).astype(NP_BF16)  # [I, NCH]
        ctm = np.ascontiguousarray(c_t[ch, :]).astype(NP_BF16)  # [NCH, I]
        lm_g = lm.reshape(NGQ, 128)                            # [g, p]
        lamb = np.ascontiguousarray(
            np.broadcast_to(lm_g[:, :, None], (NGQ, 128, TT))
            .transpose(1, 0, 2)
            .reshape(128, NGQ * TT)
            .astype(np.float32)
        )
        # lpw columns: g*TT+u -> lam_{g,p}^(u+1)  (tau=0 corrections)
        #              (NGQ+t-1)*TT+u -> lam_{HOT,p}^(t*TT+u+1)  (hot group)
        lpw = np.empty((128, (NGQ + NTQ - 1) * TT), np.float32)
        for g in range(NGQ):
            lpw[:, g * TT:(g + 1) * TT] = lm_g[g][:, None] ** (delta[None, :] + 1)
        for t in range(1, NTQ):
            lpw[:, (NGQ + t - 1) * TT:(NGQ + t) * TT] = (
                lm_g[HOT][:, None] ** (t * TT + delta[None, :] + 1)
            )
        lpw = lpw.astype(NP_BF16)
        # dcoef[p, r*NGQ+g] = lam_{g,p}^(Q*(q-1-q_r)) if s_r==s and q_r<q else 0
        dco = np.zeros((128, NCORES * NGQ), np.float64)
        for r in range(NCORES):
            qr, sr = divmod(r, NS)
            if sr == s and qr < q:
                dco[:, r * NGQ:(r + 1) * NGQ] = (lm_g ** (Q * (q - 1 - qr))).T
        in_maps.append({
            "xt": x_slabs[q],
            "wt": wt,
            "ct": ctm,
            "lamb": lamb,
            "lpw": np.ascontiguousarray(lpw),
            "dcoef": np.ascontiguousarray(dco.astype(np.float32)),
        })
    return in_maps


def _unslab(slab, ntq):
    """[ntq, 128, KI*TT] -> y_T [I, ntq*TT] f32."""
    a = slab.astype(np.float32).reshape(ntq, 128, KI, TT)
    return a.transpose(2, 1, 0, 3).reshape(I, ntq * TT)


def combine_outputs(results, xs, d_skip):
    """results: per-core {"y", "ycorr"} -> full Y [L, I] f32."""
    out = np.empty((L, I), np.float32)
    for q in range(NQ):
        acc = None
        for s in range(NS):
            r = results[q * NS + s]
            yt = _unslab(r["y"], NTQ)                       # [I, Q]
            yt[:, TT:] += _unslab(r["ycorr"], NTQ - 1)
            acc = yt if acc is None else acc + yt
        out[q * Q:(q + 1) * Q] = acc.T
    out += xs * d_skip[None, :].astype(np.float32)
    return np.ascontiguousarray(out, dtype=np.float32)


def run_on_hw(xs, lam, w_in, c_out, d_skip):
    """Returns (Y full f32 [L, I], BassKernelResults)."""
    nc = _get_nc()
    in_maps = _prep_in_maps(xs, lam, w_in, c_out)
    res = run_bass_kernel_spmd(nc, in_maps, core_ids=list(range(NCORES)))
    return combine_outputs(res.results, xs, d_skip), res


def kernel(xs, lam, w_in, c_out, d_skip):
    out, _ = run_on_hw(
        np.asarray(xs, dtype=np.float32),
        np.asarray(lam, dtype=np.float32),
        np.asarray(w_in, dtype=np.float32),
        np.asarray(c_out, dtype=np.float32),
        np.asarray(d_skip, dtype=np.float32),
    )
    return out


# revision 20
# speedup vs baseline: 1.1317x; 1.0131x over previous
"""Trainium2 Bass kernel for BaseSSMLayer (diagonal linear SSM).

Computation (exactly equivalent to the reference's associative_scan — for
broadcast lambda the non-standard cell reduces to the standard recurrence):
    U = xs @ w_in.T              # [L, N]
    h_t = lam * h_{t-1} + U_t    # linear recurrence over L
    Y = H @ c_out.T + xs * d_skip

Sharding: 4 time blocks x 2 state halves (8 cores).  Each core owns a
4096-step block and 1024 state channels (globally lambda-sorted, split into
two halves, each half sorted so only its top group g7 has long memory).
Per-core DMA is ~44 MB (vs 137 MB for pure state sharding), leaving the DMA
engines ~70% idle so transient HBM contention can't starve the PE — the
PE window is then just the 2 GEMMs at the bf16 roofline.

Cross-block state: each core runs a zero-init local scan.  The incoming
state h_init = sum_{q'<q} Lam^(Q(q-1-q')) F(q') needs the other time blocks'
final local states F, exchanged mid-kernel via an 8-core AllGather (8 KiB).
The scan superposition  h_true(d) = h_local(d) + lam^(d+1) * h_init  makes
the correction purely elementwise.  Corrections only matter for tile tau=0
(all groups, small d) and the hot group g7 (lambda in [~0.94, 1)) on later
tiles: lambda^512 < 2e-17 for every cold group.  The mm2 schedule exploits
that: phase B1 (cold groups, tiles 1..7) runs right after mm1 with no
barrier, giving the collective a ~170 us window to complete; B2 (hot group,
corrected) and the tau=0 tile run at the end.  B2's contribution goes to a
separate ycorr output summed on the host, so no PSUM group ever spans the
barrier.
"""

import numpy as np
import ml_dtypes

import concourse.tile as tile
from concourse import bacc, mybir
from concourse.bass import ts
from concourse.bass_utils import run_bass_kernel_spmd

L = 16384        # sequence length
I = 2048         # in_dim (= out dim of Y)
N = 2048         # state_dim
NCORES = 8
NQ = 4           # time blocks
NS = 2           # state halves
Q = L // NQ      # 4096 timesteps per block
NCH = N // NS    # 1024 channels per core
NGQ = NCH // 128  # 8 channel groups per core
TT = 512         # time tile
NTQ = Q // TT    # 8 time tiles per block
KI = I // 128    # 16 contraction chunks over in_dim
XC = 4           # x DMA chunking: KI split into XC chunks of KXC i-tiles
KXC = KI // XC
HOT = NGQ - 1    # index of the hot (long-memory) group

BF16 = mybir.dt.bfloat16
F32 = mybir.dt.float32
NP_BF16 = ml_dtypes.bfloat16


def _build_nc():
    nc = bacc.Bacc(
        "TRN2",
        target_bir_lowering=False,
        debug=False,
        num_devices=NCORES,
    )
    xt = nc.dram_tensor("xt", [NTQ, 128, KI * TT], BF16, kind="ExternalInput").ap()
    wt = nc.dram_tensor("wt", [I, NCH], BF16, kind="ExternalInput").ap()
    ct = nc.dram_tensor("ct", [NCH, I], BF16, kind="ExternalInput").ap()
    lamb = nc.dram_tensor("lamb", [128, NGQ * TT], F32, kind="ExternalInput").ap()
    lpw = nc.dram_tensor("lpw", [128, (NGQ + NTQ - 1) * TT], BF16, kind="ExternalInput").ap()
    dcoef = nc.dram_tensor("dcoef", [128, NCORES * NGQ], F32, kind="ExternalInput").ap()
    y = nc.dram_tensor("y", [NTQ, 128, KI * TT], BF16, kind="ExternalOutput").ap()
    ycorr = nc.dram_tensor("ycorr", [NTQ - 1, 128, KI * TT], BF16, kind="ExternalOutput").ap()

    with tile.TileContext(nc) as tc:
        with (
            tc.tile_pool(name="const", bufs=1) as const_pool,
            tc.tile_pool(name="xin", bufs=5) as x_pool,
            tc.tile_pool(name="hb", bufs=1) as hb_pool,
            tc.tile_pool(name="yst", bufs=4) as yst_pool,
            tc.tile_pool(name="sm", bufs=8) as sm_pool,
            tc.tile_pool(name="dram", bufs=2, space="DRAM") as dram_pool,
            tc.tile_pool(name="ups", bufs=2, space="PSUM") as u_psum,
            tc.tile_pool(name="yps", bufs=4, space="PSUM") as y_psum,
        ):
            # --- resident constants ---
            w_sb = []
            for i in range(KI):
                w = const_pool.tile([128, NCH], BF16, tag=f"w{i}", name=f"w{i}")
                nc.scalar.dma_start(w[:], wt[i * 128:(i + 1) * 128, :])
                w_sb.append(w)
            lam_sb = const_pool.tile([128, NGQ * TT], F32, tag="lam")
            nc.scalar.dma_start(lam_sb[:], lamb[:])
            dco_sb = const_pool.tile([128, NCORES * NGQ], F32, tag="dco")
            nc.scalar.dma_start(dco_sb[:], dcoef[:])
            lpw_sb = const_pool.tile([128, (NGQ + NTQ - 1) * TT], BF16, tag="lpw")
            nc.scalar.dma_start(lpw_sb[:], lpw[:])
            c_sb = []
            for g in range(NGQ):
                c = const_pool.tile([128, I], BF16, tag=f"c{g}", name=f"c{g}")
                nc.scalar.dma_start(c[:], ct[g * 128:(g + 1) * 128, :])
                c_sb.append(c)

            # Pre-warm the PE HAM clock gate during the initial DMA ramp.
            warm_sb = const_pool.tile([128, 128], BF16, tag="warm")
            nc.gpsimd.memset(warm_sb[:], 0.0)
            warm_ps = u_psum.tile([128, 128], F32, tag="u")
            for _ in range(16):
                nc.tensor.matmul(warm_ps[:], warm_sb[:], warm_sb[:], start=True, stop=True)

            xt3 = xt.rearrange("t p (j c) -> t p j c", j=XC)
            yo4 = y.rearrange("t p (o u) -> t p o u", o=KI)
            yc4 = ycorr.rearrange("t p (o u) -> t p o u", o=KI)

            hb_tiles = [[None] * NGQ for _ in range(NTQ)]
            hb_prev = [None] * NGQ

            # ---------------- phase A: mm1 + scans ----------------
            for t in range(NTQ):
                x_chunks = []
                for j in range(XC):
                    xc = x_pool.tile([128, KXC * TT], BF16, tag="x")
                    nc.sync.dma_start(xc[:], xt3[t, :, j])
                    x_chunks.append(xc)
                for g in range(NGQ):
                    u_ps = u_psum.tile([128, TT], F32, tag="u")
                    for i in range(KI):
                        j, ic = divmod(i, KXC)
                        nc.tensor.matmul(
                            u_ps[:],
                            w_sb[i][:, g * 128:(g + 1) * 128],
                            x_chunks[j][:, ts(ic, TT)],
                            start=(i == 0),
                            stop=(i == KI - 1),
                        )
                        if (t == 0 or (t == 1 and g < 4)) and i % 4 == 3 and i < KI - 1:
                            # keep the PE busy through the DMA-paced ramp so
                            # the HAM clock gate never re-throttles
                            for _ in range(2):
                                nc.tensor.matmul(
                                    warm_ps[:], warm_sb[:], warm_sb[:],
                                    start=True, stop=True,
                                )
                    hb = hb_pool.tile([128, TT], BF16, tag="hb", bufs=NTQ * NGQ + 2)
                    init = 0.0 if t == 0 else hb_prev[g][:, TT - 1: TT]
                    nc.vector.tensor_tensor_scan(
                        hb[:],
                        lam_sb[:, ts(g, TT)],
                        u_ps[:],
                        init,
                        op0=mybir.AluOpType.mult,
                        op1=mybir.AluOpType.add,
                    )
                    hb_prev[g] = hb
                    hb_tiles[t][g] = hb

            # ---------------- F exchange (gpsimd queue, overlaps B1) ----------
            f_sb = const_pool.tile([128, NGQ], F32, tag="fsb")
            for g in range(NGQ):
                nc.vector.tensor_copy(f_sb[:, g:g + 1], hb_tiles[NTQ - 1][g][:, TT - 1: TT])
            DISABLE_CC = True  # timing probe: skip the collective
            gath_sb = const_pool.tile([128, NCORES * NGQ], F32, tag="gath")
            if DISABLE_CC:
                nc.vector.memset(gath_sb[:], 0.0)
            else:
                cc_in = dram_pool.tile([128, NGQ], F32, tag="ccin")
                cc_out = dram_pool.tile([NCORES, 128, NGQ], F32, tag="ccout")
                nc.gpsimd.dma_start(cc_in[:], f_sb[:])
                nc.gpsimd.collective_compute(
                    "AllGather",
                    mybir.AluOpType.bypass,
                    replica_groups=[list(range(NCORES))],
                    ins=[cc_in.opt()],
                    outs=[cc_out.opt()],
                )
                for r in range(NCORES):
                    nc.gpsimd.dma_start(gath_sb[:, ts(r, NGQ)], cc_out[r])
            # h_init[p, g] = sum_r gath[r, p, g] * dcoef[r, p, g]
            acc = sm_pool.tile([128, NGQ], F32, tag="hacc")
            nc.vector.tensor_tensor(
                acc[:], gath_sb[:, ts(0, NGQ)], dco_sb[:, ts(0, NGQ)],
                op=mybir.AluOpType.mult,
            )
            for r in range(1, NCORES):
                prod = sm_pool.tile([128, NGQ], F32, tag="hprod")
                nc.vector.tensor_tensor(
                    prod[:], gath_sb[:, ts(r, NGQ)], dco_sb[:, ts(r, NGQ)],
                    op=mybir.AluOpType.mult,
                )
                acc2 = sm_pool.tile([128, NGQ], F32, tag="hacc")
                nc.vector.tensor_tensor(
                    acc2[:], acc[:], prod[:], op=mybir.AluOpType.add,
                )
                acc = acc2
            h_init = acc

            # corrections: hbc = lpw * h_init + hb_local   (gpsimd, overlaps B1)
            hbc0 = [None] * NGQ     # tau=0, all groups
            for g in range(NGQ):
                hc = hb_pool.tile([128, TT], BF16, tag="hbc", bufs=NGQ + NTQ - 1)
                nc.vector.scalar_tensor_tensor(
                    hc[:],
                    lpw_sb[:, ts(g, TT)],
                    h_init[:, g:g + 1],
                    hb_tiles[0][g][:],
                    op0=mybir.AluOpType.mult,
                    op1=mybir.AluOpType.add,
                )
                hbc0[g] = hc
            hbc7 = [None] * NTQ     # tau=1..7, hot group
            for t in range(1, NTQ):
                hc = hb_pool.tile([128, TT], BF16, tag="hbc", bufs=NGQ + NTQ - 1)
                nc.vector.scalar_tensor_tensor(
                    hc[:],
                    lpw_sb[:, ts(NGQ + t - 1, TT)],
                    h_init[:, HOT:HOT + 1],
                    hb_tiles[t][HOT][:],
                    op0=mybir.AluOpType.mult,
                    op1=mybir.AluOpType.add,
                )
                hbc7[t] = hc

            # ---------------- phase B1: cold groups, tiles 1..7 ----------------
            # Engine split avoids head-of-line blocking: ACT drains B1/tau0
            # groups (never waits on the collective), DVE drains B2 (queued
            # after h_init, which is where B2 can first run anyway), and the
            # idle-by-then sync queue issues all y DMAs.
            def emit_b1(t, b2_iter=None):
                for o in range(KI):
                    y_ps = y_psum.tile([128, TT], F32, tag="y")
                    for g in range(NGQ - 1):
                        nc.tensor.matmul(
                            y_ps[:],
                            c_sb[g][:, ts(o, 128)],
                            hb_tiles[t][g][:],
                            start=(g == 0),
                            stop=(g == NGQ - 2),
                        )
                    y_stage = yst_pool.tile([128, TT], BF16, tag="yst")
                    nc.scalar.copy(y_stage[:], y_ps[:])
                    nc.sync.dma_start(yo4[t, :, o], y_stage[:])
                    if b2_iter is not None:
                        for _ in range(3):
                            item = next(b2_iter, None)
                            if item is not None:
                                emit_b2_one(*item)

            # ---------------- phase B2: hot group (corrected), tiles 1..7 ------
            def emit_b2_one(t, o):
                y_ps = y_psum.tile([128, TT], F32, tag="y")
                nc.tensor.matmul(
                    y_ps[:],
                    c_sb[HOT][:, ts(o, 128)],
                    hbc7[t][:],
                    start=True,
                    stop=True,
                )
                y_stage = yst_pool.tile([128, TT], BF16, tag="ystc")
                nc.vector.tensor_copy(y_stage[:], y_ps[:])
                nc.sync.dma_start(yc4[t - 1, :, o], y_stage[:])

            for t in range(1, 5):
                emit_b1(t)
            b2_iter = iter([(t, o) for t in range(1, NTQ) for o in range(KI)])
            for t in range(5, NTQ):
                emit_b1(t, b2_iter)
            for item in b2_iter:
                emit_b2_one(*item)

            # ---------------- tail: tile 0 (fully corrected) -------------------
            for o in range(KI):
                y_ps = y_psum.tile([128, TT], F32, tag="y")
                for g in range(NGQ):
                    nc.tensor.matmul(
                        y_ps[:],
                        c_sb[g][:, ts(o, 128)],
                        hbc0[g][:],
                        start=(g == 0),
                        stop=(g == NGQ - 1),
                    )
                y_stage = yst_pool.tile([128, TT], BF16, tag="yst")
                nc.scalar.copy(y_stage[:], y_ps[:])
                nc.sync.dma_start(yo4[0, :, o], y_stage[:])

    nc.compile()
    return nc


_NC_CACHE = None


def _get_nc():
    global _NC_CACHE
    if _NC_CACHE is None:
        _NC_CACHE = _build_nc()
    return _NC_CACHE


_PREP_CACHE = {}


def _prep_in_maps(xs, lam, w_in, c_out):
    """Per-core inputs for the 4x2 time/state sharding."""
    # channel assignment: global lambda sort, split halves, each half ascending
    order = np.argsort(lam, kind="stable")
    halves = [order[:NCH], order[NCH:]]

    # x slabs per time block (shared by the two state-half cores of a block)
    x_slabs = []
    for q in range(NQ):
        blk = np.ascontiguousarray(xs[q * Q:(q + 1) * Q].T)    # [I, Q]
        slab = (
            blk.astype(NP_BF16)
            .reshape(KI, 128, NTQ, TT)
            .transpose(2, 1, 0, 3)
            .reshape(NTQ, 128, KI * TT)
        )
        x_slabs.append(np.ascontiguousarray(slab))

    w_t = np.ascontiguousarray(w_in.T)    # [I, N]
    c_t = np.ascontiguousarray(c_out.T)   # [N, I]

    delta = np.arange(TT, dtype=np.float64)
    in_maps = []
    for k in range(NCORES):
        q, s = divmod(k, NS)
        ch = halves[s]
        lm = lam[ch].astype(np.float64)                        # [NCH]
        wt = np.ascontiguousarray(w_t[:, ch]).astype(NP_BF16)  # [I, NCH]
        ctm = np.ascontiguousarray(c_t[ch, :]).astype(NP_BF16)  # [NCH, I]
        lm_g = lm.reshape(NGQ, 128)                            # [g, p]
        lamb = np.ascontiguousarray(
            np.broadcast_to(lm_g[:, :, None], (NGQ, 128, TT))
            .transpose(1, 0, 2)
            .reshape(128, NGQ * TT)
            .astype(np.float32)
        )
        # lpw columns: g*TT+u -> lam_{g,p}^(u+1)  (tau=0 corrections)
        #              (NGQ+t-1)*TT+u -> lam_{HOT,p}^(t*TT+u+1)  (hot group)
        lpw = np.empty((128, (NGQ + NTQ - 1) * TT), np.float32)
        for g in range(NGQ):
            lpw[:, g * TT:(g + 1) * TT] = lm_g[g][:, None] ** (delta[None, :] + 1)
        for t in range(1, NTQ):
            lpw[:, (NGQ + t - 1) * TT:(NGQ + t) * TT] = (
                lm_g[HOT][:, None] ** (t * TT + delta[None, :] + 1)
            )
        lpw = lpw.astype(NP_BF16)
        # dcoef[p, r*NGQ+g] = lam_{g,p}^(Q*(q-1-q_r)) if s_r==s and q_r<q else 0
        dco = np.zeros((128, NCORES * NGQ), np.float64)
        for r in range(NCORES):
            qr, sr = divmod(r, NS)
            if sr == s and qr < q:
                dco[:, r * NGQ:(r + 1) * NGQ] = (lm_g ** (Q * (q - 1 - qr))).T
        in_maps.append({
            "xt": x_slabs[q],
            "wt": wt,
            "ct": ctm,
            "lamb": lamb,
            "lpw": np.ascontiguousarray(lpw),
            "dcoef": np.ascontiguousarray(dco.astype(np.float32)),
        })
    return in_maps


def _unslab(slab, ntq):
    """[ntq, 128, KI*TT] -> y_T [I, ntq*TT] f32."""
    a = slab.astype(np.float32).reshape(ntq, 128, KI, TT)
    return a.transpose(2, 1, 0, 3).reshape(I, ntq * TT)


def combine_outputs(results, xs, d_skip):
    """results: per-core {"y", "ycorr"} -> full Y [L, I] f32."""
    out = np.empty((L, I), np.float32)
    for q in range(NQ):
        acc = None
        for s in range(NS):
            r = results[q * NS + s]
            yt = _unslab(r["y"], NTQ)                       # [I, Q]
            yt[:, TT:] += _unslab(r["ycorr"], NTQ - 1)
            acc = yt if acc is None else acc + yt
        out[q * Q:(q + 1) * Q] = acc.T
    out += xs * d_skip[None, :].astype(np.float32)
    return np.ascontiguousarray(out, dtype=np.float32)


def run_on_hw(xs, lam, w_in, c_out, d_skip):
    """Returns (Y full f32 [L, I], BassKernelResults)."""
    nc = _get_nc()
    in_maps = _prep_in_maps(xs, lam, w_in, c_out)
    res = run_bass_kernel_spmd(nc, in_maps, core_ids=list(range(NCORES)))
    return combine_outputs(res.results, xs, d_skip), res


def kernel(xs, lam, w_in, c_out, d_skip):
    out, _ = run_on_hw(
        np.asarray(xs, dtype=np.float32),
        np.asarray(lam, dtype=np.float32),
        np.asarray(w_in, dtype=np.float32),
        np.asarray(c_out, dtype=np.float32),
        np.asarray(d_skip, dtype=np.float32),
    )
    return out


# revision 21
# speedup vs baseline: 1.3946x; 1.2323x over previous
"""Trainium2 Bass kernel for BaseSSMLayer (diagonal linear SSM).

Computation (verified equivalent to the reference's associative_scan):
    U = xs @ w_in.T              # [L, N]
    h_t = lam * h_{t-1} + U_t    # linear recurrence over L
    Y = H @ c_out.T + xs * d_skip

Strategy: tensor-parallel over state channels (N=2048 -> 256 per core,
8 cores, no cross-core communication).  Each core works in transposed
space (channels/out-dim on SBUF partitions, time on the free axis):

    matmul1 (TensorE, bf16): U_T[n, t] = w_sh.T @ xs_T      (contraction over in_dim)
    scan    (VectorE, f32 state): H_T[n, t] = lam*H_T[n, t-1] + U_T[n, t]
                             (hardware tensor_tensor_scan along free axis)
    matmul2 (TensorE, bf16): Yp_T[o, t] = c_sh.T @ H_T      (contraction over n-shard)

The 8 partial Yp_T are summed on the host (f32) and the diagonal skip
xs * d_skip is added there too.

Data layout: xs/y live in DRAM as [NT, 128, KI*TT] "slab" blocks so each
DMA moves 16 KiB-contiguous runs per partition (descriptor-rate limits
dominate at 1 KiB).  Host does the (cheap) permutations.
"""

import numpy as np
import ml_dtypes

import concourse.tile as tile
from concourse import bacc, mybir
from concourse.bass import ts
from concourse.bass_utils import run_bass_kernel_spmd

L = 16384        # sequence length
I = 2048         # in_dim (= out dim of Y)
N = 2048         # state_dim
NCORES = 8
NSH = N // NCORES        # 256 state channels per core
NG = NSH // 128          # 2 partition-groups of channels per core
TT = 512                 # time-tile (free dim per matmul / scan)
NT = L // TT             # 32 time tiles
KI = I // 128            # 16 contraction tiles over in_dim

BF16 = mybir.dt.bfloat16
F32 = mybir.dt.float32
NP_BF16 = ml_dtypes.bfloat16


def _build_nc():
    nc = bacc.Bacc(
        "TRN2",
        target_bir_lowering=False,
        debug=False,
        num_devices=NCORES,
    )
    xt = nc.dram_tensor("xt", [NT, 128, KI * TT], BF16, kind="ExternalInput").ap()
    wt = nc.dram_tensor("wt", [I, NSH], BF16, kind="ExternalInput").ap()
    ct = nc.dram_tensor("ct", [NSH, I], BF16, kind="ExternalInput").ap()
    lamb = nc.dram_tensor("lamb", [128, NG * TT], F32, kind="ExternalInput").ap()
    y = nc.dram_tensor("y", [NT, 128, KI * TT], BF16, kind="ExternalOutput").ap()

    NCH = 4           # DMA chunking: KI split into NCH chunks of KC i-tiles
    KC = KI // NCH

    with tile.TileContext(nc) as tc:
        with (
            tc.tile_pool(name="const", bufs=1) as const_pool,
            tc.tile_pool(name="xin", bufs=4 * NCH) as x_pool,
            tc.tile_pool(name="hb", bufs=6) as hb_pool,
            tc.tile_pool(name="yst", bufs=2 * NCH) as yst_pool,
            tc.tile_pool(name="ups", bufs=2, space="PSUM") as u_psum,
            tc.tile_pool(name="yps", bufs=6, space="PSUM") as y_psum,
        ):
            # --- resident constants (w chunked so the first MMs start early) ---
            w_sb = []
            for j in range(NCH):
                w = const_pool.tile([128, KC * NSH], BF16, tag=f"w{j}")
                nc.scalar.dma_start(
                    w[:].rearrange("p (i n) -> p i n", i=KC),
                    wt[j * KC * 128:(j + 1) * KC * 128, :].rearrange(
                        "(i p) n -> p i n", p=128
                    ),
                )
                w_sb.append(w)
            lam_sb = const_pool.tile([128, NG * TT], F32, tag="lam")
            nc.scalar.dma_start(lam_sb[:], lamb[:])
            # c_out is first needed by mm2(t=0) at ~25us; issue its DMA from
            # the vector queue after the t=0 scans so it doesn't steal HBM
            # bandwidth from the critical w/x ramp.
            c_sb = [
                const_pool.tile([128, I], BF16, tag=f"c{g}", name=f"c{g}")
                for g in range(NG)
            ]

            # Pre-warm the PE HAM clock gate during the initial DMA ramp so the
            # first real matmuls run at 2.4 GHz instead of 1.2 GHz.  Keep the
            # up-front burst short (it queues ahead of the real matmuls);
            # more warm MMs are sprinkled into tile 0 to pad DMA waits.
            warm_sb = const_pool.tile([128, 128], BF16, tag="warm")
            nc.gpsimd.memset(warm_sb[:], 0.0)
            warm_ps = u_psum.tile([128, 128], F32, tag="u")
            for _ in range(16):
                nc.tensor.matmul(warm_ps[:], warm_sb[:], warm_sb[:], start=True, stop=True)

            hb_prev = [None] * NG          # bf16 H tile of previous slab, per group
            hb_tiles = [None] * (NT * NG)  # bf16 H tiles pending matmul2
            xt3 = xt.rearrange("t p (j c) -> t p j c", j=NCH)

            def emit_mm1(t):
                x_chunks = []
                for j in range(NCH):
                    xc = x_pool.tile([128, KC * TT], BF16, tag="x")
                    nc.sync.dma_start(xc[:], xt3[t, :, j])
                    x_chunks.append(xc)

                def mm1_one(u_ps, g, i):
                    j, ic = divmod(i, KC)
                    nc.tensor.matmul(
                        u_ps[:],
                        w_sb[j][:, ic * NSH + g * 128: ic * NSH + (g + 1) * 128],
                        x_chunks[j][:, ts(ic, TT)],
                        start=(i == 0),
                        stop=(i == KI - 1),
                    )

                u_list = []
                for g in range(NG):
                    u_ps = u_psum.tile([128, TT], F32, tag="u")
                    u_list.append(u_ps)
                if t == 0:
                    for i in range(KI):
                        for g in range(NG):
                            mm1_one(u_list[g], g, i)
                        # pad the DMA-paced ramp with warm matmuls so the PE
                        # never idles long enough for the HAM to re-throttle
                        if i % 4 == 3 and i < KI - 1:
                            for _ in range(2):
                                nc.tensor.matmul(
                                    warm_ps[:], warm_sb[:], warm_sb[:],
                                    start=True, stop=True,
                                )
                else:
                    for g in range(NG):
                        for i in range(KI):
                            mm1_one(u_list[g], g, i)
                for g in range(NG):
                    u_ps = u_list[g]
                    hb = hb_pool.tile([128, TT], BF16, tag="hb")
                    init = 0.0 if t == 0 else hb_prev[g][:, TT - 1: TT]
                    nc.vector.tensor_tensor_scan(
                        hb[:],
                        lam_sb[:, ts(g, TT)],
                        u_ps[:],
                        init,
                        op0=mybir.AluOpType.mult,
                        op1=mybir.AluOpType.add,
                    )
                    hb_prev[g] = hb
                    hb_tiles[t * NG + g] = hb

            def emit_mm2(t):
                last = t == NT - 1
                for j in range(NCH):
                    y_stage = yst_pool.tile([128, KC * TT], BF16, tag="yst")
                    for oc in range(KC):
                        o = j * KC + oc
                        y_ps = y_psum.tile([128, TT], F32, tag="y")
                        for g in range(NG):
                            nc.tensor.matmul(
                                y_ps[:],
                                c_sb[g][:, ts(o, 128)],
                                hb_tiles[t * NG + g][:],
                                start=(g == 0),
                                stop=(g == NG - 1),
                            )
                        # drain PSUM -> bf16 staging; split across ACT and DVE
                        if oc in (0, KC - 1):
                            nc.vector.tensor_copy(y_stage[:, ts(oc, TT)], y_ps[:])
                        else:
                            nc.scalar.copy(y_stage[:, ts(oc, TT)], y_ps[:])
                        if last:
                            # fine-grained drain of the final slab so the tail
                            # DMA overlaps the remaining copies
                            nc.scalar.dma_start(
                                yo4[t, :, j, oc], y_stage[:, ts(oc, TT)]
                            )
                    if not last:
                        nc.scalar.dma_start(yo3[t, :, j], y_stage[:])

            yo3 = y.rearrange("t p (j c) -> t p j c", j=NCH)
            yo4 = y.rearrange("t p (j o u) -> t p j o u", j=NCH, o=KC)

            # software-pipelined: matmul2 for slab t runs one slab behind
            # matmul1, so the PE never waits on the scan chain.
            for t in range(NT + 1):
                if t < NT:
                    emit_mm1(t)
                if t == 1:
                    # c_out lands on the sync queue behind x(t0)+x(t1): off
                    # the critical ramp, well before mm2(0) needs it (~22us)
                    for g in range(NG):
                        nc.sync.dma_start(c_sb[g][:], ct[g * 128:(g + 1) * 128, :])
                if t >= 1:
                    emit_mm2(t - 1)

    nc.compile()
    return nc


_NC_CACHE = None


def _get_nc():
    global _NC_CACHE
    if _NC_CACHE is None:
        _NC_CACHE = _build_nc()
    return _NC_CACHE


def _prep_in_maps(xs, lam, w_in, c_out):
    # xs.T -> [KI, 128, NT, TT] -> [NT, 128, KI, TT] slabs (16 KiB runs/partition)
    xt = (
        np.ascontiguousarray(xs.T)
        .astype(NP_BF16)
        .reshape(KI, 128, NT, TT)
        .transpose(2, 1, 0, 3)
        .reshape(NT, 128, KI * TT)
    )
    xt = np.ascontiguousarray(xt)
    w_t = np.ascontiguousarray(w_in.T)                        # [I, N]
    c_t = np.ascontiguousarray(c_out.T)                       # [N, I]
    in_maps = []
    for k in range(NCORES):
        sh = slice(k * NSH, (k + 1) * NSH)
        wt = np.ascontiguousarray(w_t[:, sh]).astype(NP_BF16)     # [I, NSH]
        ct = np.ascontiguousarray(c_t[sh, :]).astype(NP_BF16)     # [NSH, I]
        lam_sh = lam[sh].reshape(NG, 128).astype(np.float32)      # [g, p]
        lamb = np.ascontiguousarray(
            np.broadcast_to(lam_sh[:, :, None], (NG, 128, TT))
            .transpose(1, 0, 2)
            .reshape(128, NG * TT)
        )
        in_maps.append({"xt": xt, "wt": wt, "ct": ct, "lamb": lamb})
    return in_maps


def combine_outputs(results, xs, d_skip):
    """results: list of per-core {"y": [NT, 128, KI*TT] bf16} -> full Y [L, I] f32."""
    acc = results[0]["y"].astype(np.float32)
    for r in results[1:]:
        acc += r["y"].astype(np.float32)
    # [NT, 128, KI, TT] -> Y_T [I, L] -> Y [L, I]
    y_t = acc.reshape(NT, 128, KI, TT).transpose(2, 1, 0, 3).reshape(I, L)
    out = y_t.T + xs * d_skip[None, :].astype(np.float32)
    return np.ascontiguousarray(out, dtype=np.float32)


def run_on_hw(xs, lam, w_in, c_out, d_skip):
    """Returns (Y full f32 [L, I], BassKernelResults)."""
    nc = _get_nc()
    in_maps = _prep_in_maps(xs, lam, w_in, c_out)
    res = run_bass_kernel_spmd(nc, in_maps, core_ids=list(range(NCORES)))
    return combine_outputs(res.results, xs, d_skip), res


def kernel(xs, lam, w_in, c_out, d_skip):
    out, _ = run_on_hw(
        np.asarray(xs, dtype=np.float32),
        np.asarray(lam, dtype=np.float32),
        np.asarray(w_in, dtype=np.float32),
        np.asarray(c_out, dtype=np.float32),
        np.asarray(d_skip, dtype=np.float32),
    )
    return out

